# revision 4
# baseline (speedup 1.0000x reference)
"""Causal multi-head attention on 8 TRN2 NeuronCores.

Reference (per batch b):
    q,k,v = x @ W^T  (W: [d_out, d_in]), split into H=16 heads of dk=64
    attn  = softmax(causal(q k^T / sqrt(dk))) v
    y     = concat_heads(attn) @ W_o^T

Sharding (8 cores): core c -> batch b = c//4, head group g = c%4 (4 heads,
256 channels).  w_q/w_k/w_v column-sharded by head, w_o row-sharded - each
core computes a partial y[b] over its 256 channels; the host sums the 4
partials per batch (the unshard step).  y partials are written bf16 (host
accumulates in f32; ~+0.2% on a ~0.6% rel-err budget, gate 2e-2).

Per-core engine load (bf16, warm 2.4 GHz): PE streams ~270k column-cycles
(~112us: qk-proj 65.5k, v-proj 32.8k, scores 69.6k, PV 69.6k, y-proj
32.8k) - the PSUM-drain-port floor for this decomposition; ScalarE exp
~81us; DVE ~70us.  PE is the bottleneck, so everything else is organized
around keeping its queue dense:

  - Emission interleaves at pair granularity: between each scores-pair and
    its PV (which waits ~2.3us for ScalarE exp), a "filler" PE group is
    emitted - a projection group for the NEXT s-chunk or an output-proj
    s-tile for the PREVIOUS chunk.  Hides exp latency, keeps HAM at 2.4.
  - All DRAM tensors are staged by the host in SBUF-native layout
    ([partition, ...] with >=2KB contiguous per partition line) - strided
    128-partition rearranges of the logical layouts run the DMA engines at
    a fraction of peak (256B lines measured ~4x slower end-to-end).
  - Head: x chunk 0 moves first as 4 pieces on the scalar/gpsimd queues,
    w_k/w_q lead the sync queue, w_v/w_o ride scalar/gpsimd, and the
    first projection groups are emitted in dependency order (k-et0, q-et0,
    v, with k/q-et1 as fillers inside attention chunk 0).
  - Tail: the last chunk's hp=1 softmax normalization is split into two
    256-col halves, each immediately followed by its two output-proj
    s-tiles, so the final y matmuls/copies/DMAs pipeline with the
    reciprocal chain instead of trailing it.

Layout choices (as the 160us baseline):
  - All matmul inputs bf16 (host-cast), f32 PSUM accumulation.
  - x staged transposed (x^T: [d, s]) so q^T/k^T ([e_local, s]) come
    straight out of the PE and serve as lhsT/rhs of the scores matmul.
  - Scores computed transposed: S^T[kpos, q], two heads packed in the
    128-row PE array via row tiling, one 2-bank PSUM pair per kpos-tile,
    one ScalarE ACTIVATE does exp for both heads.
  - Softmax without max-subtraction (scores are O(10), exp safe in f32).
  - Causality at tile granularity: fully-masked kpos-tiles skipped,
    diagonal tiles shift/shrink to the valid q-range, residual mask is one
    128-col triangle (f >= p) multiply per diagonal tile.
  - P @ V via V_aug = [V | 1]: row 64 of the accumulated [65, q] output is
    the softmax denominator.
  - attn^T = out[0:64] * (1/denominator) via DVE fast reciprocal + gpsimd
    partition broadcast (custom ops need base-partition-0 SBUF inputs).
"""

from functools import partial

import numpy as np
import ml_dtypes

B = 2
S = 2048
D = 1024
H = 16
DK = 64
NCORES = 8
EL = 256  # local channels per core (4 heads)
QW = 512  # q-chunk width (free dim of scores matmuls)
NJ = S // QW  # 4 q-chunks

_CACHE = {}


def _build():
    import concourse.bass as bass
    import concourse.mybir as mybir
    import concourse.tile as tile
    from concourse import bacc

    f32 = mybir.dt.float32
    bf16 = mybir.dt.bfloat16
    ts = bass.ts
    Exp = mybir.ActivationFunctionType.Exp

    nc = bacc.Bacc("TRN2", num_devices=NCORES)
    DT = D // 128  # 8 d-tiles
    ST = S // 128  # 16 s-tiles

    # host-packed, SBUF-native layouts (partition-major, contiguous lines)
    xT_d = nc.dram_tensor("xT", [128, NJ * DT * QW], bf16, kind="ExternalInput")
    wqT_d = nc.dram_tensor("wqT", [128, DT * EL], bf16, kind="ExternalInput")
    wkT_d = nc.dram_tensor("wkT", [128, DT * EL], bf16, kind="ExternalInput")
    wvT_d = nc.dram_tensor("wvT", [128, DT * EL], bf16, kind="ExternalInput")
    woT_d = nc.dram_tensor("woT", [128, (EL // 128) * D], bf16, kind="ExternalInput")
    y_d = nc.dram_tensor("y", [128, ST * D], bf16, kind="ExternalOutput")

    with tile.TileContext(nc) as tc:
        with (
            tc.tile_pool(name="big", bufs=1) as big,
            tc.tile_pool(name="work", bufs=3) as work,
            tc.tile_pool(name="psum", bufs=1, space="PSUM") as psum,
        ):
            xT = big.tile([128, NJ, DT, QW], bf16)  # x^T (d%128, chunk, d//128, s)
            wqT = big.tile([128, DT, EL], bf16)
            wkT = big.tile([128, DT, EL], bf16)
            wvT = big.tile([128, DT, EL], bf16)
            woT = big.tile([128, EL // 128, D], bf16)
            qT = big.tile([128, 2, S], bf16)  # (e_local, s)
            kT = big.tile([128, 2, S], bf16)
            vA = big.tile([128, ST, 4, DK + 1], bf16)  # (s%128, s//128, h, dv+1)
            aT = big.tile([128, 2, S], bf16)  # attn^T (d_local, s)
            masks = big.tile([128, 1, 128], bf16)  # triangle: f >= p

            # ---- input DMAs.  x chunk 0 first (4 pieces, scalar+gpsimd
            # queues), w_k/w_q lead the sync queue, w_v/w_o on the scalar/
            # gpsimd queues behind the x pieces, x chunks 1-3 as one DMA.
            xT_r = xT_d.ap().rearrange("p (c k s) -> p c k s", c=NJ, k=DT)
            nc.scalar.dma_start(xT[:, 0, 0:2, :], xT_r[:, 0, 0:2, :])
            nc.gpsimd.dma_start(xT[:, 0, 2:4, :], xT_r[:, 0, 2:4, :])
            nc.scalar.dma_start(xT[:, 0, 4:6, :], xT_r[:, 0, 4:6, :])
            nc.gpsimd.dma_start(xT[:, 0, 6:8, :], xT_r[:, 0, 6:8, :])
            nc.sync.dma_start(
                wkT[:], wkT_d.ap().rearrange("p (k e) -> p k e", k=DT)
            )
            nc.sync.dma_start(
                wqT[:], wqT_d.ap().rearrange("p (k e) -> p k e", k=DT)
            )
            nc.scalar.dma_start(
                wvT[:], wvT_d.ap().rearrange("p (k e) -> p k e", k=DT)
            )
            nc.gpsimd.dma_start(
                woT[:], woT_d.ap().rearrange("p (k e) -> p k e", k=EL // 128)
            )
            nc.sync.dma_start(xT[:, 1:NJ, :, :], xT_r[:, 1:NJ, :, :])

            # ---- constants: triangle mask + V_aug ones ----
            nc.gpsimd.memset(masks[:], 1.0)
            nc.gpsimd.affine_select(
                out=masks[:, 0, :],
                in_=masks[:, 0, :],
                compare_op=mybir.AluOpType.is_ge,
                fill=0.0,
                base=0,
                pattern=[[1, 128]],
                channel_multiplier=-1,
            )
            nc.gpsimd.memset(vA[:, :, :, DK], 1.0)

            # ---- PE warmup: a few short dummy matmuls during the DMA wait
            # nudge HAM toward 2.4 GHz without delaying the first real group.
            warm = work.tile([128, EL], bf16, tag="warm", bufs=1)
            nc.vector.memset(warm[:], 0.0)
            for g in range(6):
                wp = psum.tile([128, EL], f32, tag="mm", bufs=2)
                nc.tensor.matmul(wp[:], warm[:, 0:128], warm[:])

            def qk_group(sj, which, et):
                w_sb, outT = (wkT, kT) if which == 0 else (wqT, qT)
                ps = psum.tile([128, QW], f32, tag="mm", bufs=2)
                for kd in range(DT):
                    nc.tensor.matmul(
                        ps[:],
                        w_sb[:, kd, ts(et, 128)],
                        xT[:, sj, kd, :],
                        start=(kd == 0),
                        stop=(kd == DT - 1),
                    )
                nc.vector.tensor_copy(outT[:, et, ts(sj, QW)], ps[:])

            def v_group(st):
                ps = psum.tile([128, EL], f32, tag="mm", bufs=2)
                for kd in range(DT):
                    nc.tensor.matmul(
                        ps[:],
                        xT[:, st // 4, kd, ts(st % 4, 128)],
                        wvT[:, kd, :],
                        start=(kd == 0),
                        stop=(kd == DT - 1),
                    )
                nc.vector.tensor_copy(
                    vA[:, st, :, 0:DK],
                    ps[:].rearrange("p (h e) -> p h e", h=4),
                )

            y_r = y_d.ap().rearrange("p (so e) -> p so e", so=ST)
            dmaq = [nc.sync, nc.scalar, nc.gpsimd]

            def sd_st(jd, st, alt=0):
                # one s-tile of y = attn^T.T @ w_o^T: two 512-col psum
                # groups staged into one [128, 1024] bf16 tile, one DMA
                # (2KB per partition line).
                yt = work.tile([128, D], bf16, tag="yout", bufs=3)
                for eo in range(2):
                    ps = psum.tile([128, QW], f32, tag="mm", bufs=2)
                    for kd in range(EL // 128):
                        nc.tensor.matmul(
                            ps[:],
                            aT[:, kd, ts(st, 128)],
                            woT[:, kd, ts(eo, QW)],
                            start=(kd == 0),
                            stop=(kd == EL // 128 - 1),
                        )
                    # tail pieces alternate DVE/ACT (both idle there);
                    # mid-kernel pieces stay on DVE
                    if alt % 2 == 1:
                        nc.scalar.copy(yt[:, ts(eo, QW)], ps[:])
                    else:
                        nc.vector.tensor_copy(yt[:, ts(eo, QW)], ps[:])
                q = dmaq[alt % 3] if jd == NJ - 1 else nc.sync
                q.dma_start(y_r[:, st, :], yt[:])

            fillers = []

            def run_filler():
                if fillers:
                    fillers.pop(0)()

            def norm_head(j, hp, hh, oa, c0, cw):
                # 1/denominator for head 2*hp+hh over chunk-j columns
                # [c0, c0+cw); returns the broadcast tile
                dn = work.tile([1, QW], f32, tag="dn", bufs=3)
                nc.vector.tensor_copy(dn[:, 0:cw], oa[DK : DK + 1, bass.ds(c0, cw)])
                rc = work.tile([1, QW], f32, tag="rc", bufs=3)
                nc.vector.reciprocal_approx_fast(out=rc[:, 0:cw], in_=dn[:, 0:cw])
                bc = work.tile([64, QW], f32, tag="bc", bufs=3)
                nc.gpsimd.partition_broadcast(bc[:, 0:cw], rc[:, 0:cw])
                return bc

            def norm_mul(j, hp, hh, oa, bc, c0, cw):
                h = 2 * hp + hh
                nc.vector.tensor_mul(
                    aT[(h % 2) * 64 : (h % 2) * 64 + 64, h // 2, bass.ds(j * QW + c0, cw)],
                    oa[0:DK, bass.ds(c0, cw)],
                    bc[:, 0:cw],
                )

            def attn_chunk(j):
                last = j == NJ - 1
                ilast = 4 * j + 3
                for hp in range(2):
                    oa0 = psum.tile([128, QW], f32, tag="oa", bufs=2)
                    oa1 = psum.tile([128, QW], f32, tag="oa", bufs=2)

                    def geom(i):
                        # diagonal tiles only need q >= kpos: shift the
                        # q-range by 128r and shrink; the remaining mask is
                        # always the 128-col triangle f>=p.
                        if i >= 4 * j:
                            off = 128 * (i - 4 * j)
                            return off, QW - off
                        return 0, QW

                    for ii in range(0, ilast + 1, 2):
                        pair = (ii, ii + 1)
                        scs = {}
                        for i in pair:
                            off, NW = geom(i)
                            sc = psum.tile([128, 2 * QW], f32, tag="sc", bufs=2)
                            nc.tensor.matmul(
                                sc[:, 0:NW],
                                kT[0:64, hp, ts(i, 128)],
                                qT[0:64, hp, bass.ds(j * QW + off, NW)],
                                tile_position=(0, 0),
                            )
                            nc.tensor.matmul(
                                sc[:, QW : QW + NW],
                                kT[64:128, hp, ts(i, 128)],
                                qT[64:128, hp, bass.ds(j * QW + off, NW)],
                                tile_position=(64, 0),
                            )
                            scs[i] = sc
                        # filler PE group lands between the scores pair and
                        # its PV, hiding the ScalarE exp latency
                        run_filler()
                        eos = {}
                        for i in pair:
                            off, NW = geom(i)
                            e01 = work.tile(
                                [128, 2 * QW], bf16, tag="exps", bufs=8
                            )
                            sc_v = scs[i][:].rearrange("p (h q) -> p h q", h=2)
                            e_v = e01[:].rearrange("p (h q) -> p h q", h=2)
                            nc.scalar.activation(
                                e_v[:, :, 0:NW], sc_v[:, :, 0:NW], Exp, scale=0.125
                            )
                            if i >= 4 * j:  # diagonal: mask first 128 cols
                                nc.vector.tensor_mul(
                                    e_v[:, :, 0:128],
                                    e_v[:, :, 0:128],
                                    masks[:, 0:1, :].to_broadcast((128, 2, 128)),
                                )
                            eos[i] = e01
                        for i in pair:
                            off, NW = geom(i)
                            nc.tensor.matmul(
                                oa0[0 : DK + 1, off : off + NW],
                                vA[:, i, 2 * hp, :],
                                eos[i][:, 0:NW],
                                start=(i == 0),
                                stop=(i == ilast),
                            )
                            nc.tensor.matmul(
                                oa1[0 : DK + 1, off : off + NW],
                                vA[:, i, 2 * hp + 1, :],
                                eos[i][:, QW : QW + NW],
                                start=(i == 0),
                                stop=(i == ilast),
                            )
                    # softmax normalization.  Last chunk hp=1: split into
                    # 256-col halves, each followed by its two output-proj
                    # s-tiles, so the tail pipelines instead of chaining.
                    if last and hp == 1:
                        hw = QW // 2
                        for half in range(2):
                            bcs = {}
                            for hh, oa in ((0, oa0), (1, oa1)):
                                bcs[hh] = norm_head(j, hp, hh, oa, half * hw, hw)
                            for hh, oa in ((0, oa0), (1, oa1)):
                                norm_mul(j, hp, hh, oa, bcs[hh], half * hw, hw)
                            for st in (4 * j + 2 * half, 4 * j + 2 * half + 1):
                                sd_st(j, st, alt=st)
                    else:
                        bcs = {}
                        for hh, oa in ((0, oa0), (1, oa1)):
                            bcs[hh] = norm_head(j, hp, hh, oa, 0, QW)
                        for hh, oa in ((0, oa0), (1, oa1)):
                            norm_mul(j, hp, hh, oa, bcs[hh], 0, QW)

            # ---- the pipeline ----
            # chunk 0 projections in dependency order; k/q et=1 become the
            # first fillers (attention chunk 0 hp=0 only needs et=0)
            qk_group(0, 0, 0)
            qk_group(0, 1, 0)
            for st in range(4):
                v_group(st)
            fillers.append(partial(qk_group, 0, 0, 1))
            fillers.append(partial(qk_group, 0, 1, 1))

            for sj in range(NJ):
                # filler jobs consumed inside attn_chunk(sj): next chunk's
                # projections (k/q et0 first - attn(sj+1) hp0 needs them)
                # interleaved with the previous chunk's output projection
                jobs = []
                if sj + 1 < NJ:
                    jobs += [
                        partial(qk_group, sj + 1, 0, 0),
                        partial(qk_group, sj + 1, 1, 0),
                        partial(v_group, 4 * (sj + 1) + 0),
                        partial(v_group, 4 * (sj + 1) + 1),
                        partial(qk_group, sj + 1, 0, 1),
                        partial(qk_group, sj + 1, 1, 1),
                        partial(v_group, 4 * (sj + 1) + 2),
                        partial(v_group, 4 * (sj + 1) + 3),
                    ]
                if sj >= 1:
                    sd = [
                        partial(sd_st, sj - 1, 4 * (sj - 1) + st)
                        for st in range(4)
                    ]
                    mixed = []
                    for idx, a in enumerate(jobs):
                        mixed.append(a)
                        if idx % 2 == 1 and sd:
                            mixed.append(sd.pop(0))
                    mixed += sd
                    jobs = mixed
                fillers.extend(jobs)
                attn_chunk(sj)
                while fillers:
                    run_filler()

    nc.compile()
    return nc


def _get_nc():
    if "nc" not in _CACHE:
        _CACHE["nc"] = _build()
    return _CACHE["nc"]


def _pack_x(xb):
    # x[b] [S, D] f32 -> [128, NJ, DT, QW] partition-major bf16, flattened
    DT = D // 128
    xT = xb.T.reshape(DT, 128, NJ, QW)
    return np.ascontiguousarray(xT.transpose(1, 2, 0, 3).reshape(128, -1)).astype(
        ml_dtypes.bfloat16
    )


def _pack_w(wT):
    # w^T [D, e] -> [128, D//128, e] partition-major bf16, flattened
    ko = wT.shape[0] // 128
    return np.ascontiguousarray(
        wT.reshape(ko, 128, -1).transpose(1, 0, 2).reshape(128, -1)
    ).astype(ml_dtypes.bfloat16)


def kernel(x, w_q, w_k, w_v, w_o, _trace=False, _trace_cores=None):
    from concourse.bass_utils import run_bass_kernel_spmd

    nc = _get_nc()
    in_maps = []
    for c in range(NCORES):
        b = c // 4
        g = c % 4
        ch = slice(g * EL, (g + 1) * EL)
        in_maps.append(
            {
                "xT": _pack_x(x[b]),
                "wqT": _pack_w(w_q[ch, :].T),
                "wkT": _pack_w(w_k[ch, :].T),
                "wvT": _pack_w(w_v[ch, :].T),
                "woT": _pack_w(w_o[:, ch].T),
            }
        )
    res = run_bass_kernel_spmd(
        nc,
        in_maps,
        core_ids=list(range(NCORES)),
        trace=_trace,
        trace_cores=_trace_cores,
    )
    _CACHE["last_results"] = res
    y = np.zeros((B, S, D), np.float32)
    ST = S // 128
    for c in range(NCORES):
        yp = res.results[c]["y"].reshape(128, ST, D).astype(np.float32)
        y[c // 4] += yp.transpose(1, 0, 2).reshape(S, D)
    return y


# revision 8
# speedup vs baseline: 1.0005x; 1.0005x over previous
"""Causal multi-head attention on 8 TRN2 NeuronCores.

Reference (per batch b):
    q,k,v = x @ W^T  (W: [d_out, d_in]), split into H=16 heads of dk=64
    attn  = softmax(causal(q k^T / sqrt(dk))) v
    y     = concat_heads(attn) @ W_o^T

Sharding (8 cores): core c -> batch b = c//4, head group g = c%4 (4 heads,
256 channels).  w_q/w_k/w_v column-sharded by head, w_o row-sharded - each
core computes a partial y[b] over its 256 channels; the host sums the 4
partials per batch (the unshard step).  y partials are written bf16 (host
accumulates in f32; ~+0.2% on a ~0.6% rel-err budget, gate 2e-2).

Per-core engine load (bf16, warm 2.4 GHz): PE streams ~270k column-cycles
(~112us: qk-proj 65.5k, v-proj 32.8k, scores 69.6k, PV 69.6k, y-proj
32.8k) - the PSUM-drain-port floor for this decomposition; ScalarE exp
~81us; DVE ~70us.  PE is the bottleneck, so everything else is organized
around keeping its queue dense:

  - Emission interleaves at pair granularity: between each scores-pair and
    its PV (which waits ~2.3us for ScalarE exp), a "filler" PE group is
    emitted - a projection group for the NEXT s-chunk or an output-proj
    s-tile for the PREVIOUS chunk.  Hides exp latency, keeps HAM at 2.4.
  - All DRAM tensors are staged by the host in SBUF-native layout
    ([partition, ...] with >=2KB contiguous per partition line) - strided
    128-partition rearranges of the logical layouts run the DMA engines at
    a fraction of peak (256B lines measured ~4x slower end-to-end).
  - Head: x chunk 0 moves first as 4 pieces on the scalar/gpsimd queues,
    w_k/w_q lead the sync queue, w_v/w_o ride scalar/gpsimd, and the
    first projection groups are emitted in dependency order (k-et0, q-et0,
    v, with k/q-et1 as fillers inside attention chunk 0).
  - Tail: the last chunk's hp=1 softmax normalization is split into two
    256-col halves, each immediately followed by its two output-proj
    s-tiles, so the final y matmuls/copies/DMAs pipeline with the
    reciprocal chain instead of trailing it.

Layout choices (as the 160us baseline):
  - All matmul inputs bf16 (host-cast), f32 PSUM accumulation.
  - x staged transposed (x^T: [d, s]) so q^T/k^T ([e_local, s]) come
    straight out of the PE and serve as lhsT/rhs of the scores matmul.
  - Scores computed transposed: S^T[kpos, q], two heads packed in the
    128-row PE array via row tiling, one 2-bank PSUM pair per kpos-tile,
    one ScalarE ACTIVATE does exp for both heads.
  - Softmax without max-subtraction (scores are O(10), exp safe in f32).
  - Causality at tile granularity: fully-masked kpos-tiles skipped,
    diagonal tiles shift/shrink to the valid q-range, residual mask is one
    128-col triangle (f >= p) multiply per diagonal tile.
  - P @ V via V_aug = [V | 1]: row 64 of the accumulated [65, q] output is
    the softmax denominator.
  - attn^T = out[0:64] * (1/denominator) via DVE fast reciprocal + gpsimd
    partition broadcast (custom ops need base-partition-0 SBUF inputs).
"""

from functools import partial

import numpy as np
import ml_dtypes

B = 2
S = 2048
D = 1024
H = 16
DK = 64
NCORES = 8
EL = 256  # local channels per core (4 heads)
QW = 512  # q-chunk width (free dim of scores matmuls)
NJ = S // QW  # 4 q-chunks

_CACHE = {}


def _build():
    import concourse.bass as bass
    import concourse.mybir as mybir
    import concourse.tile as tile
    from concourse import bacc

    f32 = mybir.dt.float32
    bf16 = mybir.dt.bfloat16
    ts = bass.ts
    Exp = mybir.ActivationFunctionType.Exp

    nc = bacc.Bacc("TRN2", num_devices=NCORES)
    DT = D // 128  # 8 d-tiles
    ST = S // 128  # 16 s-tiles

    # host-packed, SBUF-native layouts (partition-major, contiguous lines)
    xT_d = nc.dram_tensor("xT", [128, NJ * DT * QW], bf16, kind="ExternalInput")
    wqT_d = nc.dram_tensor("wqT", [128, DT * EL], bf16, kind="ExternalInput")
    wkT_d = nc.dram_tensor("wkT", [128, DT * EL], bf16, kind="ExternalInput")
    wvT_d = nc.dram_tensor("wvT", [128, DT * EL], bf16, kind="ExternalInput")
    woT_d = nc.dram_tensor("woT", [128, (EL // 128) * D], bf16, kind="ExternalInput")
    y_d = nc.dram_tensor("y", [128, ST * D], bf16, kind="ExternalOutput")

    with tile.TileContext(nc) as tc:
        with (
            tc.tile_pool(name="big", bufs=1) as big,
            tc.tile_pool(name="work", bufs=3) as work,
            tc.tile_pool(name="psum", bufs=1, space="PSUM") as psum,
        ):
            xT = big.tile([128, NJ, DT, QW], bf16)  # x^T (d%128, chunk, d//128, s)
            wqT = big.tile([128, DT, EL], bf16)
            wkT = big.tile([128, DT, EL], bf16)
            wvT = big.tile([128, DT, EL], bf16)
            woT = big.tile([128, EL // 128, D], bf16)
            qT = big.tile([128, 2, S], bf16)  # (e_local, s)
            kT = big.tile([128, 2, S], bf16)
            vA = big.tile([128, ST, 4, DK + 1], bf16)  # (s%128, s//128, h, dv+1)
            aT = big.tile([128, 2, S], bf16)  # attn^T (d_local, s)
            masks = big.tile([128, 1, 128], bf16)  # triangle: f >= p

            # ---- input DMAs.  x chunk 0 first (4 pieces, scalar+gpsimd
            # queues), w_k/w_q lead the sync queue, w_v/w_o on the scalar/
            # gpsimd queues behind the x pieces, x chunks 1-3 as one DMA.
            # later x chunks ride BEHIND the critical transfers on the same
            # queues - a separate early jumbo would win DMA arbitration and
            # starve x chunk 0 (measured: +8us on the first projections)
            xT_r = xT_d.ap().rearrange("p (c k s) -> p c k s", c=NJ, k=DT)
            nc.scalar.dma_start(xT[:, 0, 0:2, :], xT_r[:, 0, 0:2, :])
            nc.gpsimd.dma_start(xT[:, 0, 2:4, :], xT_r[:, 0, 2:4, :])
            nc.scalar.dma_start(xT[:, 0, 4:6, :], xT_r[:, 0, 4:6, :])
            nc.gpsimd.dma_start(xT[:, 0, 6:8, :], xT_r[:, 0, 6:8, :])
            nc.sync.dma_start(
                wkT[:], wkT_d.ap().rearrange("p (k e) -> p k e", k=DT)
            )
            nc.sync.dma_start(
                wqT[:], wqT_d.ap().rearrange("p (k e) -> p k e", k=DT)
            )
            nc.scalar.dma_start(
                wvT[:], wvT_d.ap().rearrange("p (k e) -> p k e", k=DT)
            )
            nc.gpsimd.dma_start(
                woT[:], woT_d.ap().rearrange("p (k e) -> p k e", k=EL // 128)
            )
            nc.sync.dma_start(xT[:, 1, :, :], xT_r[:, 1, :, :])
            nc.scalar.dma_start(xT[:, 2, :, :], xT_r[:, 2, :, :])
            nc.gpsimd.dma_start(xT[:, 3, :, :], xT_r[:, 3, :, :])

            # ---- constants: triangle mask + V_aug ones ----
            nc.gpsimd.memset(masks[:], 1.0)
            nc.gpsimd.affine_select(
                out=masks[:, 0, :],
                in_=masks[:, 0, :],
                compare_op=mybir.AluOpType.is_ge,
                fill=0.0,
                base=0,
                pattern=[[1, 128]],
                channel_multiplier=-1,
            )
            nc.gpsimd.memset(vA[:, :, :, DK], 1.0)

            # ---- PE warmup: a few short dummy matmuls during the DMA wait
            # nudge HAM toward 2.4 GHz without delaying the first real group.
            warm = work.tile([128, EL], bf16, tag="warm", bufs=1)
            nc.vector.memset(warm[:], 0.0)
            for g in range(6):
                wp = psum.tile([128, EL], f32, tag="mm", bufs=2)
                nc.tensor.matmul(wp[:], warm[:, 0:128], warm[:])

            def qk_group(sj, which, et):
                w_sb, outT = (wkT, kT) if which == 0 else (wqT, qT)
                ps = psum.tile([128, QW], f32, tag="mm", bufs=2)
                for kd in range(DT):
                    nc.tensor.matmul(
                        ps[:],
                        w_sb[:, kd, ts(et, 128)],
                        xT[:, sj, kd, :],
                        start=(kd == 0),
                        stop=(kd == DT - 1),
                    )
                nc.vector.tensor_copy(outT[:, et, ts(sj, QW)], ps[:])

            def v_group(st):
                ps = psum.tile([128, EL], f32, tag="mm", bufs=2)
                for kd in range(DT):
                    nc.tensor.matmul(
                        ps[:],
                        xT[:, st // 4, kd, ts(st % 4, 128)],
                        wvT[:, kd, :],
                        start=(kd == 0),
                        stop=(kd == DT - 1),
                    )
                nc.vector.tensor_copy(
                    vA[:, st, :, 0:DK],
                    ps[:].rearrange("p (h e) -> p h e", h=4),
                )

            y_r = y_d.ap().rearrange("p (so e) -> p so e", so=ST)
            dmaq = [nc.sync, nc.scalar, nc.gpsimd]

            def sd_st(jd, st, alt=0):
                # one s-tile of y = attn^T.T @ w_o^T: two 512-col psum
                # groups staged into one [128, 1024] bf16 tile, one DMA
                # (2KB per partition line).
                yt = work.tile([128, D], bf16, tag="yout", bufs=3)
                for eo in range(2):
                    ps = psum.tile([128, QW], f32, tag="mm", bufs=2)
                    for kd in range(EL // 128):
                        nc.tensor.matmul(
                            ps[:],
                            aT[:, kd, ts(st, 128)],
                            woT[:, kd, ts(eo, QW)],
                            start=(kd == 0),
                            stop=(kd == EL // 128 - 1),
                        )
                    # tail pieces alternate DVE/ACT (both idle there);
                    # mid-kernel pieces stay on DVE
                    if alt % 2 == 1:
                        nc.scalar.copy(yt[:, ts(eo, QW)], ps[:])
                    else:
                        nc.vector.tensor_copy(yt[:, ts(eo, QW)], ps[:])
                q = dmaq[alt % 3] if jd == NJ - 1 else nc.sync
                q.dma_start(y_r[:, st, :], yt[:])

            fillers = []

            def run_filler():
                if fillers:
                    fillers.pop(0)()

            def norm_head(j, hp, hh, oa, c0, cw):
                # 1/denominator for head 2*hp+hh over chunk-j columns
                # [c0, c0+cw); returns the broadcast tile
                dn = work.tile([1, QW], f32, tag="dn", bufs=3)
                nc.vector.tensor_copy(dn[:, 0:cw], oa[DK : DK + 1, bass.ds(c0, cw)])
                rc = work.tile([1, QW], f32, tag="rc", bufs=3)
                nc.vector.reciprocal_approx_fast(out=rc[:, 0:cw], in_=dn[:, 0:cw])
                bc = work.tile([64, QW], f32, tag="bc", bufs=3)
                nc.gpsimd.partition_broadcast(bc[:, 0:cw], rc[:, 0:cw])
                return bc

            def norm_mul(j, hp, hh, oa, bc, c0, cw):
                h = 2 * hp + hh
                nc.vector.tensor_mul(
                    aT[(h % 2) * 64 : (h % 2) * 64 + 64, h // 2, bass.ds(j * QW + c0, cw)],
                    oa[0:DK, bass.ds(c0, cw)],
                    bc[:, 0:cw],
                )

            def attn_chunk(j, tail_jobs=()):
                last = j == NJ - 1
                ilast = 4 * j + 3
                for hp in range(2):
                    oa0 = psum.tile([128, QW], f32, tag="oa", bufs=2)
                    oa1 = psum.tile([128, QW], f32, tag="oa", bufs=2)

                    def geom(i):
                        # diagonal tiles only need q >= kpos: shift the
                        # q-range by 128r and shrink; the remaining mask is
                        # always the 128-col triangle f>=p.
                        if i >= 4 * j:
                            off = 128 * (i - 4 * j)
                            return off, QW - off
                        return 0, QW

                    for ii in range(0, ilast + 1, 2):
                        pair = (ii, ii + 1)
                        scs = {}
                        for i in pair:
                            off, NW = geom(i)
                            sc = psum.tile([128, 2 * QW], f32, tag="sc", bufs=2)
                            nc.tensor.matmul(
                                sc[:, 0:NW],
                                kT[0:64, hp, ts(i, 128)],
                                qT[0:64, hp, bass.ds(j * QW + off, NW)],
                                tile_position=(0, 0),
                            )
                            nc.tensor.matmul(
                                sc[:, QW : QW + NW],
                                kT[64:128, hp, ts(i, 128)],
                                qT[64:128, hp, bass.ds(j * QW + off, NW)],
                                tile_position=(64, 0),
                            )
                            scs[i] = sc
                        # filler PE group lands between the scores pair and
                        # its PV, hiding the ScalarE exp latency
                        run_filler()
                        eos = {}
                        for i in pair:
                            off, NW = geom(i)
                            e01 = work.tile(
                                [128, 2 * QW], bf16, tag="exps", bufs=8
                            )
                            sc_v = scs[i][:].rearrange("p (h q) -> p h q", h=2)
                            e_v = e01[:].rearrange("p (h q) -> p h q", h=2)
                            nc.scalar.activation(
                                e_v[:, :, 0:NW], sc_v[:, :, 0:NW], Exp, scale=0.125
                            )
                            if i >= 4 * j:  # diagonal: mask first 128 cols
                                nc.vector.tensor_mul(
                                    e_v[:, :, 0:128],
                                    e_v[:, :, 0:128],
                                    masks[:, 0:1, :].to_broadcast((128, 2, 128)),
                                )
                            eos[i] = e01
                        for i in pair:
                            off, NW = geom(i)
                            nc.tensor.matmul(
                                oa0[0 : DK + 1, off : off + NW],
                                vA[:, i, 2 * hp, :],
                                eos[i][:, 0:NW],
                                start=(i == 0),
                                stop=(i == ilast),
                            )
                            nc.tensor.matmul(
                                oa1[0 : DK + 1, off : off + NW],
                                vA[:, i, 2 * hp + 1, :],
                                eos[i][:, QW : QW + NW],
                                start=(i == 0),
                                stop=(i == ilast),
                            )
                    # softmax normalization.  Last chunk hp=1: split into
                    # 256-col halves, each followed by its two output-proj
                    # s-tiles, so the tail pipelines instead of chaining.
                    # tail_jobs (reserved prev-chunk y-proj pieces) run
                    # first - they keep the PE busy and HAM at 2.4 GHz
                    # through the otherwise PE-idle reciprocal chain.
                    if last and hp == 1:
                        for tj in tail_jobs:
                            tj()
                        hw = QW // 2
                        for half in range(2):
                            bcs = {}
                            for hh, oa in ((0, oa0), (1, oa1)):
                                bcs[hh] = norm_head(j, hp, hh, oa, half * hw, hw)
                            for hh, oa in ((0, oa0), (1, oa1)):
                                norm_mul(j, hp, hh, oa, bcs[hh], half * hw, hw)
                            for st in (4 * j + 2 * half, 4 * j + 2 * half + 1):
                                sd_st(j, st, alt=st)
                    else:
                        bcs = {}
                        for hh, oa in ((0, oa0), (1, oa1)):
                            bcs[hh] = norm_head(j, hp, hh, oa, 0, QW)
                        for hh, oa in ((0, oa0), (1, oa1)):
                            norm_mul(j, hp, hh, oa, bcs[hh], 0, QW)

            # ---- the pipeline ----
            # chunk 0 projections in dependency order; k/q et=1 become the
            # first fillers (attention chunk 0 hp=0 only needs et=0)
            qk_group(0, 0, 0)
            qk_group(0, 1, 0)
            for st in range(4):
                v_group(st)
            fillers.append(partial(qk_group, 0, 0, 1))
            fillers.append(partial(qk_group, 0, 1, 1))

            for sj in range(NJ):
                # filler jobs consumed inside attn_chunk(sj): next chunk's
                # projections (k/q et0 first - attn(sj+1) hp0 needs them)
                # interleaved with the previous chunk's output projection
                jobs = []
                if sj + 1 < NJ:
                    jobs += [
                        partial(qk_group, sj + 1, 0, 0),
                        partial(qk_group, sj + 1, 1, 0),
                        partial(v_group, 4 * (sj + 1) + 0),
                        partial(v_group, 4 * (sj + 1) + 1),
                        partial(qk_group, sj + 1, 0, 1),
                        partial(qk_group, sj + 1, 1, 1),
                        partial(v_group, 4 * (sj + 1) + 2),
                        partial(v_group, 4 * (sj + 1) + 3),
                    ]
                tail_jobs = ()
                if sj >= 1:
                    sd = [
                        partial(sd_st, sj - 1, 4 * (sj - 1) + st)
                        for st in range(4)
                    ]
                    if sj == NJ - 1:
                        # reserve the last two as tail fillers
                        sd, tail_jobs = sd[:2], tuple(sd[2:])
                    mixed = []
                    for idx, a in enumerate(jobs):
                        mixed.append(a)
                        if idx % 2 == 1 and sd:
                            mixed.append(sd.pop(0))
                    mixed += sd
                    jobs = mixed
                fillers.extend(jobs)
                attn_chunk(sj, tail_jobs)
                while fillers:
                    run_filler()

    nc.compile()
    return nc


def _get_nc():
    if "nc" not in _CACHE:
        _CACHE["nc"] = _build()
    return _CACHE["nc"]


def _pack_x(xb):
    # x[b] [S, D] f32 -> [128, NJ, DT, QW] partition-major bf16, flattened
    DT = D // 128
    xT = xb.T.reshape(DT, 128, NJ, QW)
    return np.ascontiguousarray(xT.transpose(1, 2, 0, 3).reshape(128, -1)).astype(
        ml_dtypes.bfloat16
    )


def _pack_w(wT):
    # w^T [D, e] -> [128, D//128, e] partition-major bf16, flattened
    ko = wT.shape[0] // 128
    return np.ascontiguousarray(
        wT.reshape(ko, 128, -1).transpose(1, 0, 2).reshape(128, -1)
    ).astype(ml_dtypes.bfloat16)


def kernel(x, w_q, w_k, w_v, w_o, _trace=False, _trace_cores=None):
    from concourse.bass_utils import run_bass_kernel_spmd

    nc = _get_nc()
    in_maps = []
    for c in range(NCORES):
        b = c // 4
        g = c % 4
        ch = slice(g * EL, (g + 1) * EL)
        in_maps.append(
            {
                "xT": _pack_x(x[b]),
                "wqT": _pack_w(w_q[ch, :].T),
                "wkT": _pack_w(w_k[ch, :].T),
                "wvT": _pack_w(w_v[ch, :].T),
                "woT": _pack_w(w_o[:, ch].T),
            }
        )
    res = run_bass_kernel_spmd(
        nc,
        in_maps,
        core_ids=list(range(NCORES)),
        trace=_trace,
        trace_cores=_trace_cores,
    )
    _CACHE["last_results"] = res
    y = np.zeros((B, S, D), np.float32)
    ST = S // 128
    for c in range(NCORES):
        yp = res.results[c]["y"].reshape(128, ST, D).astype(np.float32)
        y[c // 4] += yp.transpose(1, 0, 2).reshape(S, D)
    return y
